# revision 52
# baseline (speedup 1.0000x reference)
"""ArcFace loss kernel for 8 TRN2 NeuronCores.

Reference computation (see problem statement):
    w_n   = weight / max(||weight_row||, 1e-12)            # [C, D]
    cos   = emb @ w_n.T                                    # [B, C]
    logit = SCALE * cos;  logit[b, lab[b]] -= SCALE*MARGIN
    loss  = mean_b( logsumexp(logit[b]) - logit[b, lab[b]] )

Sharding: classes (C=100000) split over 8 cores (12500 each), embeddings +
labels replicated.  Per core, fused in one NEFF:
  - weight shard is streamed in, row-normalized (sum-of-squares on the scalar
    engine via Square+accum, rsqrt via quake-trick + Newton on the vector
    engine, SCALE folded in), cast to bf16 on gpsimd, transposed 128x128 at a
    time on the tensor engine to produce w_n^T tiles for the matmul,
  - logits are accumulated in PSUM groups of [128b x 1536c] (bf16 matmul,
    fp32 accumulate); per-group row-max on the vector engine (negated, to
    serve as the exp bias), exp + row-sum in one scalar-engine op reading
    PSUM in place,
  - label logits come from an indirect-DMA row gather of the weight shard,
  - cross-core softmax reduction is one small AllGather of per-core
    (row-max, row-sumexp, label-logit) stats; every core redundantly
    combines them with free-axis vector math,
  - final per-row loss and the batch mean (partition-reduce via a ones
    matmul) are computed on every core; core 0's scalar is returned.

The Bacc lowering (`nc.compile()`) splits semaphore waits that exceed the
per-instruction ISA sync-slot limits; the explicit ldweights "absorber"
instructions and ACT/DVE-side copies keep hot-loop instructions within one
foreign wait each so that pass stays cheap.
"""

import numpy as np
from contextlib import ExitStack

B = 1024
D = 512
C = 100000
NCORES = 8
C_LOC = C // NCORES          # 12500
SCALE = 30.0
MARGIN = 0.5
SM = SCALE * MARGIN          # 15.0
CHUNK = 1536                 # classes per PSUM stats group (3 fp32 banks)

RSQRT_MAGIC = 0x5F3759DF


def _dve_rsqrt_scale(nc, pool, x, n, scale_out, bufs=12):
    """out = scale_out / sqrt(x), elementwise on a [128, n] fp32 SBUF tile.

    Quake initial guess + 2 Newton iterations on the vector engine (the
    scalar engine's Rsqrt is banned, and Sqrt lives in a different activation
    table set than Exp/Ln — switching sets costs ~2.7us each time).
    """
    import concourse.mybir as mybir

    f32 = mybir.dt.float32
    i32 = mybir.dt.int32
    y = pool.tile([128, n], f32, tag="rsq_y", bufs=bufs, name="rsq_y")
    t = pool.tile([128, n], f32, tag="rsq_t", bufs=bufs, name="rsq_t")
    xc = pool.tile([128, n], f32, tag="rsq_x", bufs=bufs, name="rsq_x")
    # clamp away from 0 (shard-padding rows have sum-of-squares 0; matches
    # the reference's max(norm, eps) in spirit)
    nc.vector.tensor_scalar(
        out=xc, in0=x, scalar1=1e-12, scalar2=None, op0=mybir.AluOpType.max)
    x = xc
    xi = x.bitcast(i32)
    yi = y.bitcast(i32)
    # y0 bits = MAGIC - (xi >> 1)  ==  (~(xi >> 1)) + (MAGIC + 1)
    nc.vector.tensor_scalar(
        out=yi, in0=xi, scalar1=1, scalar2=-1,
        op0=mybir.AluOpType.arith_shift_right, op1=mybir.AluOpType.bitwise_xor,
    )
    nc.vector.tensor_scalar(
        out=yi, in0=yi, scalar1=RSQRT_MAGIC + 1, scalar2=None,
        op0=mybir.AluOpType.add,
    )
    for it in range(2):
        nc.vector.tensor_tensor(out=t, in0=y, in1=y, op=mybir.AluOpType.mult)
        nc.vector.tensor_tensor(out=t, in0=t, in1=x, op=mybir.AluOpType.mult)
        # t = 1.5 - 0.5 * x * y^2
        nc.vector.tensor_scalar(
            out=t, in0=t, scalar1=-0.5, scalar2=1.5,
            op0=mybir.AluOpType.mult, op1=mybir.AluOpType.add,
        )
        if it == 1:
            nc.vector.tensor_scalar(
                out=t, in0=t, scalar1=float(scale_out), scalar2=None,
                op0=mybir.AluOpType.mult,
            )
        nc.vector.tensor_tensor(out=y, in0=y, in1=t, op=mybir.AluOpType.mult)
    return y


def build_nc(b=B, d=D, c_loc=C_LOC, chunk=CHUNK, n_cores=NCORES, debug=False):
    import concourse.bass as bass
    import concourse.tile as tile
    import concourse.mybir as mybir
    from concourse.masks import make_identity

    f32 = mybir.dt.float32
    bf16 = mybir.dt.bfloat16
    i32 = mybir.dt.int32
    Alu = mybir.AluOpType
    Act = mybir.ActivationFunctionType

    nb = b // 128            # batch tiles
    nd = d // 128            # contraction tiles
    c_pad = ((c_loc + 127) // 128) * 128
    groups = []
    c0 = 0
    while c0 < c_pad:
        gw = min(chunk, c_pad - c0)
        groups.append((c0, gw))
        c0 += gw
    ng = len(groups)

    if debug:
        nc = bass.Bass(target_bir_lowering=False, debug=True)
    else:
        from concourse import bacc
        nc = bacc.Bacc()

    # the weight shard arrives zero-padded to a multiple of 128 rows
    emb = nc.declare_dram_parameter("emb", [b, d], f32, isOutput=False)
    wsh = nc.declare_dram_parameter("wsh", [c_pad, d], f32, isOutput=False)
    lab = nc.declare_dram_parameter("lab", [b], i32, isOutput=False)
    out = nc.declare_dram_parameter("out", [1], f32, isOutput=True)

    with ExitStack() as ctx:
        tc = ctx.enter_context(tile.TileContext(nc))
        dram = ctx.enter_context(tc.tile_pool(name="dram", bufs=1, space="DRAM"))
        res = ctx.enter_context(tc.tile_pool(name="res", bufs=1))
        work = ctx.enter_context(tc.tile_pool(name="work", bufs=3))
        wtp = ctx.enter_context(tc.tile_pool(name="wtp", bufs=3))
        psum = ctx.enter_context(tc.tile_pool(name="psum", bufs=2, space="PSUM"))

        # collective bounce buffers (internal DRAM)
        st_in = dram.tile([128, 3 * nb], f32, tag="st_in", name="st_in")
        st_out = dram.tile([n_cores * 128, 3 * nb], f32, tag="st_out",
                           name="st_out", addr_space="Shared")

        ident = res.tile([128, 128], bf16, tag="ident", name="ident")
        make_identity(nc, ident)

        # ---------------- Phase A: embeddings ----------------
        e_all = res.tile([128, nb, d], f32, tag="e_all", name="e_all")
        nc.sync.dma_start(out=e_all, in_=emb.rearrange("(t p) d -> p t d", p=128))
        e_b16 = res.tile([128, nb, d], bf16, tag="e_b16", name="e_b16")
        nc.vector.tensor_copy(out=e_b16, in_=e_all)
        eT = res.tile([128, nd, b], bf16, tag="eT", name="eT")
        for t in range(nb):
            tp = psum.tile([128, d], bf16, tag="tp", name="tp", space="PSUM")
            # dummy weight-load absorbs the newest DVE tick (input data for
            # fresh slots, the recycled slot's last evacuation otherwise) on
            # the PE's clock, so the transposes below carry at most one
            # semaphore wait (the matmul ISA struct fits 1 wait + 1 update)
            if t < 2:
                nc.tensor.ldweights(e_b16[:, t, 0:128])
            else:
                nc.tensor.ldweights(eT[:, 0, (t - 2) * 128:(t - 1) * 128])
            for dd in range(nd):
                nc.tensor.transpose(
                    tp[:, dd * 128:(dd + 1) * 128],
                    e_b16[:, t, dd * 128:(dd + 1) * 128], ident)
            nc.vector.tensor_copy(
                out=eT[:, :, t * 128:(t + 1) * 128],
                in_=tp.rearrange("p (a q) -> p a q", a=nd))
        e_f = [e_all[:, t, :] for t in range(nb)]

        # ---------------- Phase E: label logits ----------------
        lab_s = res.tile([128, nb], i32, tag="lab_s", name="lab_s")
        nc.sync.dma_start(out=lab_s, in_=lab.rearrange("(t p) -> p t", p=128))
        labf = res.tile([128, nb], f32, tag="labf", name="labf")
        nc.vector.tensor_copy(out=labf, in_=lab_s)
        offs = res.tile([128, nb], i32, tag="offs", name="offs")
        nc.vector.tensor_scalar(
            out=offs, in0=lab_s, scalar1=0, scalar2=c_loc - 1,
            op0=Alu.max, op1=Alu.min)
        T_st = res.tile([128, nb], f32, tag="T_st", name="T_st")
        ssg = res.tile([128, nb], f32, tag="ssg", name="ssg")
        ldot = res.tile([128, nb], f32, tag="ldot", name="ldot")
        # one indirect DMA gathers all label rows (keeps SWDGE lanes fresh)
        wg_all = res.tile([128, nb, d], f32, tag="wg_all", name="wg_all")
        nc.gpsimd.indirect_dma_start(
            out=wg_all, out_offset=None, in_=wsh[:, :],
            in_offset=bass.IndirectOffsetOnAxis(ap=offs[:, :], axis=0),
        )
        lab_junk2 = res.tile([128, nb, d], bf16, tag="lab_junk2", name="lab_junk2")
        lab_junk = res.tile([128, nb, d], bf16, tag="lab_junk", name="lab_junk")
        for t in range(nb):
            nc.vector.tensor_tensor(
                out=lab_junk2[:, t, :], in0=wg_all[:, t, :], in1=e_f[t],
                op=Alu.mult)
            nc.vector.tensor_reduce(
                out=ldot[:, t:t + 1], in_=lab_junk2[:, t, :],
                axis=mybir.AxisListType.X, op=Alu.add)
        for t in range(nb):
            # elementwise output is junk (fresh bf16 slices, no hazards);
            # only the f32 accum matters
            nc.scalar.activation(
                out=lab_junk[:, t, :], in_=wg_all[:, t, :], func=Act.Square,
                accum_out=ssg[:, t:t + 1])
        linv = _dve_rsqrt_scale(nc, res, ssg, nb, SCALE)
        nc.vector.tensor_tensor(out=T_st, in0=ldot, in1=linv, op=Alu.mult)
        # mask out labels that don't belong to this core's shard
        msk = res.tile([128, nb], f32, tag="msk", name="msk")
        nc.vector.tensor_scalar(
            out=msk, in0=labf, scalar1=0.0, scalar2=None, op0=Alu.is_ge)
        nc.vector.tensor_tensor(out=T_st, in0=T_st, in1=msk, op=Alu.mult)
        nc.vector.tensor_scalar(
            out=msk, in0=labf, scalar1=float(c_loc - 1), scalar2=None,
            op0=Alu.is_le)
        nc.vector.tensor_tensor(out=T_st, in0=T_st, in1=msk, op=Alu.mult)

        # ---------------- stats tiles ----------------
        nm_st = []
        nm2_st = []
        s_st = []
        for t in range(nb):
            nmt = res.tile([128, ng], f32, tag=f"nm{t}", name=f"nm{t}")
            nm2t = res.tile([128, ng], f32, tag=f"nm2_{t}", name=f"nm2_{t}")
            sst = res.tile([128, ng], f32, tag=f"ss{t}", name=f"ss{t}")
            nm_st.append(nmt)
            nm2_st.append(nm2t)
            s_st.append(sst)

        # ---------------- Phases B/C/D per group ----------------
        # tiny per-sub DVE probe reads absorb the load-DMA semaphore so the
        # normalize ops below only carry their (mandatory) same-engine wait
        probe_all = res.tile([128, 64], f32, tag="probe", name="probe_all")
        probe_idx = [0]
        SUB = 6  # w blocks per f32 staging tile (SBUF pressure)
        for g, (g0, gw) in enumerate(groups):
            nblk = gw // 128
            # B: load f32 sub-tiles, sum-of-squares (ACT Square+accum — the
            # junk elementwise output goes into the wtb slice that the
            # normalize overwrites later, so it adds no scratch and no WAW
            # semaphore), rsqrt (DVE), normalize+cast to bf16 (DVE).
            # Shard-padding rows are zero: squares 0, rsqrt clamped, w_n 0.
            ssq = work.tile([128, nblk], f32, tag="ssq", bufs=2, name="ssq")
            wtb = work.tile([128, nblk, d], bf16, tag="wtb", bufs=3, name="wtb")
            subs = []
            for s0 in range(0, nblk, SUB):
                sblk = min(SUB, nblk - s0)
                wtg = work.tile([128, SUB, d], f32, tag="wtg", bufs=3, name="wtg")
                nc.sync.dma_start(
                    out=wtg[:, :sblk, :],
                    in_=wsh[g0 + s0 * 128:g0 + (s0 + sblk) * 128, :]
                    .rearrange("(k p) d -> p k d", p=128))
                pi = probe_idx[0]
                probe_idx[0] += 1
                assert pi < 64
                nc.vector.tensor_copy(out=probe_all[:, pi:pi + 1],
                                      in_=wtg[:, 0, 0:1])
                subs.append((s0, sblk, wtg))
                for kk in range(sblk):
                    k = s0 + kk
                    nc.scalar.activation(
                        out=wtb[:, k, :], in_=wtg[:, kk, :], func=Act.Square,
                        accum_out=ssq[:, k:k + 1])
            invs = _dve_rsqrt_scale(nc, work, ssq, nblk, SCALE)
            for s0, sblk, wtg in subs:
                for kk in range(sblk):
                    k = s0 + kk
                    nc.vector.tensor_scalar(
                        out=wtb[:, k, :], in0=wtg[:, kk, :],
                        scalar1=invs[:, k:k + 1], scalar2=None, op0=Alu.mult)

            # C: transpose each 128-class block on the tensor engine
            wT = wtp.tile([128, nd, gw], bf16, tag="wT", name="wT")
            for k in range(nblk):
                tp = psum.tile([128, d], bf16, tag="tp", name="tp", space="PSUM")
                # wait absorber: target the newest DVE product (see eT loop)
                if k < 2:
                    nc.tensor.ldweights(wtb[:, k, 0:128])
                else:
                    nc.tensor.ldweights(wT[:, 0, (k - 2) * 128:(k - 1) * 128])
                for dd in range(nd):
                    nc.tensor.transpose(
                        tp[:, dd * 128:(dd + 1) * 128],
                        wtb[:, k, dd * 128:(dd + 1) * 128], ident)
                nc.vector.tensor_copy(
                    out=wT[:, :, k * 128:(k + 1) * 128],
                    in_=tp.rearrange("p (a q) -> p a q", a=nd))

            # D: matmul + stats per batch tile
            nsub = (gw + 511) // 512
            for t in range(nb):
                pt = psum.tile([128, chunk], f32, tag="pt", name="pt",
                               space="PSUM")
                if t == 0:
                    # absorb the group's last evacuation tick
                    nc.tensor.ldweights(wT[:, 0, (nblk - 1) * 128:nblk * 128])
                for s in range(nsub):
                    n0 = s * 512
                    nw = min(512, gw - n0)
                    for dd in range(nd):
                        nc.tensor.matmul(
                            pt[:, n0:n0 + nw],
                            eT[:, dd, t * 128:(t + 1) * 128],
                            wT[:, dd, n0:n0 + nw],
                            start=(dd == 0), stop=(dd == nd - 1),
                        )
                nc.vector.tensor_reduce(
                    out=nm_st[t][:, g:g + 1], in_=pt[:, :gw],
                    axis=mybir.AxisListType.X, op=Alu.max, negate=True)
                # route the bias through an ACT-side copy so the Exp below
                # only has to semaphore-wait on the tensor engine (the ACT
                # ISA struct has a single usable wait slot); the copy also
                # makes ACT observe the DVE tick, absorbing the exp's WAR
                # on the reduce_max's PSUM read
                nc.scalar.copy(out=nm2_st[t][:, g:g + 1],
                               in_=nm_st[t][:, g:g + 1])
                nc.scalar.activation(
                    out=pt[:, :gw], in_=pt[:, :gw], func=Act.Exp,
                    bias=nm2_st[t][:, g:g + 1], scale=1.0,
                    accum_out=s_st[t][:, g:g + 1])

        # ---------------- Phase F: local combine + collectives ----------------
        m_all = res.tile([128, nb], f32, tag="m_all", name="m_all")
        mk_neg = res.tile([128, nb], f32, tag="mk_neg", name="mk_neg")
        s_all = res.tile([128, nb], f32, tag="s_all", name="s_all")
        for t in range(nb):
            # nm_st holds negated group maxes; local max m_k = -min(nm)
            nc.vector.tensor_reduce(
                out=mk_neg[:, t:t + 1], in_=nm_st[t],
                axis=mybir.AxisListType.X, op=Alu.min)
            nc.vector.tensor_reduce(
                out=m_all[:, t:t + 1], in_=nm_st[t],
                axis=mybir.AxisListType.X, op=Alu.min, negate=True)
            ex = work.tile([128, ng], f32, tag="ex", bufs=2, name="ex")
            # exp(m_g - m_k) = exp(-nm_g + mk_neg)
            nc.scalar.activation(
                out=ex, in_=nm_st[t], func=Act.Exp,
                bias=mk_neg[:, t:t + 1], scale=-1.0)
            nc.vector.tensor_tensor(out=ex, in0=ex, in1=s_st[t], op=Alu.mult)
            nc.vector.tensor_reduce(
                out=s_all[:, t:t + 1], in_=ex,
                axis=mybir.AxisListType.X, op=Alu.add)

        # single AllGather of (m_k, s_k, t_k): output lands core-major on
        # the partition axis; the DMA back rearranges it so the cross-core
        # reduction becomes ordinary free-axis vector math
        stpack = res.tile([128, 3 * nb], f32, tag="stpack", name="stpack")
        nc.vector.tensor_copy(out=stpack[:, 0:nb], in_=m_all)
        nc.vector.tensor_copy(out=stpack[:, nb:2 * nb], in_=s_all)
        nc.vector.tensor_copy(out=stpack[:, 2 * nb:3 * nb], in_=T_st)
        nc.sync.dma_start(out=st_in[:, :], in_=stpack)
        nc.gpsimd.collective_compute(
            "AllGather", Alu.bypass,
            replica_groups=[list(range(n_cores))],
            ins=[st_in[:, :]], outs=[st_out[:, :]],
        )
        AG = res.tile([128, n_cores, 3 * nb], f32, tag="AG", name="AG")
        nc.sync.dma_start(
            out=AG, in_=st_out[:, :].rearrange("(k p) c -> p k c", p=128))
        AG2 = res.tile([128, n_cores, 3 * nb], f32, tag="AG2", name="AG2")
        nc.vector.tensor_copy(out=AG2, in_=AG)
        Mk = AG2[:, :, 0:nb]
        Sk = AG2[:, :, nb:2 * nb]
        Tk = AG2[:, :, 2 * nb:3 * nb]
        M2 = res.tile([128, nb], f32, tag="M2", name="M2")
        nc.vector.tensor_reduce(
            out=M2, in_=Mk.rearrange("p k c -> p c k"),
            axis=mybir.AxisListType.X, op=Alu.max)
        M2n = res.tile([128, nb], f32, tag="M2n", name="M2n")
        nc.vector.tensor_reduce(
            out=M2n, in_=Mk.rearrange("p k c -> p c k"),
            axis=mybir.AxisListType.X, op=Alu.max, negate=True)
        # s_k * exp(m_k - M), summed over cores
        dif = res.tile([128, n_cores, nb], f32, tag="dif", name="dif")
        for k in range(n_cores):
            nc.vector.tensor_tensor(
                out=dif[:, k, :], in0=Mk[:, k, :], in1=M2n, op=Alu.add)
        exd = res.tile([128, n_cores, nb], f32, tag="exd", name="exd")
        nc.scalar.activation(out=exd, in_=dif, func=Act.Exp)
        exd2 = res.tile([128, n_cores, nb], f32, tag="exd2", name="exd2")
        nc.vector.tensor_tensor(out=exd2, in0=exd, in1=Sk, op=Alu.mult)
        SGt = res.tile([128, nb], f32, tag="SGt", name="SGt")
        nc.vector.tensor_reduce(
            out=SGt, in_=exd2.rearrange("p k c -> p c k"),
            axis=mybir.AxisListType.X, op=Alu.add)
        TGt = res.tile([128, nb], f32, tag="TGt", name="TGt")
        nc.vector.tensor_reduce(
            out=TGt, in_=Tk.rearrange("p k c -> p c k"),
            axis=mybir.AxisListType.X, op=Alu.add)
        SG = SGt[:, :]
        TG = TGt[:, :]

        # loss_b = M + log(S - exp(T-M) + exp(T-M-SM)) - T + SM
        tmd = res.tile([128, nb], f32, tag="tmd", name="tmd")
        nc.vector.tensor_tensor(out=tmd, in0=TG, in1=M2, op=Alu.subtract)
        ea = res.tile([128, nb], f32, tag="ea", name="ea")
        nc.scalar.activation(out=ea, in_=tmd, func=Act.Exp)
        nsm = res.tile([128, 1], f32, tag="nsm", name="nsm")
        nc.vector.memset(nsm, -SM)
        eb = res.tile([128, nb], f32, tag="eb", name="eb")
        nc.scalar.activation(out=eb, in_=tmd, func=Act.Exp, bias=nsm[:, 0:1])
        ea2 = res.tile([128, nb], f32, tag="ea2", name="ea2")
        nc.vector.tensor_copy(out=ea2, in_=ea)
        eb2 = res.tile([128, nb], f32, tag="eb2", name="eb2")
        nc.vector.tensor_copy(out=eb2, in_=eb)
        S2 = res.tile([128, nb], f32, tag="S2", name="S2")
        nc.vector.tensor_tensor(out=S2, in0=SG, in1=ea2, op=Alu.subtract)
        nc.vector.tensor_tensor(out=S2, in0=S2, in1=eb2, op=Alu.add)
        lg = res.tile([128, nb], f32, tag="lg", name="lg")
        nc.scalar.activation(out=lg, in_=S2, func=Act.Ln)
        lg2 = res.tile([128, nb], f32, tag="lg2", name="lg2")
        nc.vector.tensor_copy(out=lg2, in_=lg)
        nc.vector.tensor_tensor(out=lg2, in0=lg2, in1=M2, op=Alu.add)
        nc.vector.tensor_tensor(out=lg2, in0=lg2, in1=TG, op=Alu.subtract)
        nc.vector.tensor_scalar(
            out=lg2, in0=lg2, scalar1=SM, scalar2=None, op0=Alu.add)

        rs = res.tile([128, 1], f32, tag="rs", name="rs")
        nc.vector.tensor_reduce(
            out=rs, in_=lg2, axis=mybir.AxisListType.X, op=Alu.add)
        # route rs through ACT so the final matmul's two deps (operand +
        # PSUM-slot release, both ACT) merge into a single semaphore wait;
        # the ones-vector is the pre-barrier const AP (no wait at all)
        rs2 = res.tile([128, 1], f32, tag="rs2", name="rs2")
        nc.scalar.copy(out=rs2, in_=rs)
        ones = nc.const_aps.tensor(1.0, (128, 1))
        fin = psum.tile([1, 1], f32, tag="pt", name="fin", space="PSUM")
        nc.tensor.matmul(fin, rs2, ones, start=True, stop=True)
        out_sb = res.tile([1, 1], f32, tag="out_sb", name="out_sb")
        nc.vector.tensor_scalar(
            out=out_sb, in0=fin[0:1, 0:1], scalar1=1.0 / b, scalar2=None,
            op0=Alu.mult)
        # SWDGE store on a fresh lane: carries only the DVE data wait
        nc.gpsimd.dma_start(out=out[0:1], in_=out_sb[0:1, 0])

    if not debug:
        # bacc lowering: splits over-capacity semaphore waits into
        # event-semaphore instructions (hardware sync-slot limits)
        nc.compile()
    return nc


def kernel(embeddings, labels, weight):
    import concourse.bass_utils as bass_utils

    emb = np.ascontiguousarray(np.asarray(embeddings, dtype=np.float32))
    labv = np.asarray(labels).astype(np.int64)
    w = np.asarray(weight, dtype=np.float32)

    nc = build_nc()
    c_pad = ((C_LOC + 127) // 128) * 128
    in_maps = []
    for k in range(NCORES):
        wpad = np.zeros((c_pad, D), dtype=np.float32)
        wpad[:C_LOC] = w[k * C_LOC:(k + 1) * C_LOC]
        in_maps.append({
            "emb": emb,
            "wsh": wpad,
            "lab": (labv - k * C_LOC).astype(np.int32),
        })
    res = bass_utils.run_bass_kernel_spmd(nc, in_maps, core_ids=list(range(NCORES)))
    return np.float32(np.asarray(res.results[0]["out"]).ravel()[0])


# revision 54
# speedup vs baseline: 1.0266x; 1.0266x over previous
"""ArcFace loss kernel for 8 TRN2 NeuronCores.

Reference computation (see problem statement):
    w_n   = weight / max(||weight_row||, 1e-12)            # [C, D]
    cos   = emb @ w_n.T                                    # [B, C]
    logit = SCALE * cos;  logit[b, lab[b]] -= SCALE*MARGIN
    loss  = mean_b( logsumexp(logit[b]) - logit[b, lab[b]] )

Sharding: classes (C=100000) split over 8 cores (12500 each), embeddings +
labels replicated.  Per core, fused in one NEFF:
  - weight shard is streamed in, row-normalized (sum-of-squares on the scalar
    engine via Square+accum, rsqrt via quake-trick + Newton on the vector
    engine, SCALE folded in), cast to bf16 on gpsimd, transposed 128x128 at a
    time on the tensor engine to produce w_n^T tiles for the matmul,
  - logits are accumulated in PSUM groups of [128b x 1536c] (bf16 matmul,
    fp32 accumulate); per-group row-max on the vector engine (negated, to
    serve as the exp bias), exp + row-sum in one scalar-engine op reading
    PSUM in place,
  - label logits come from an indirect-DMA row gather of the weight shard,
  - cross-core softmax reduction is one small AllGather of per-core
    (row-max, row-sumexp, label-logit) stats; every core redundantly
    combines them with free-axis vector math,
  - final per-row loss and the batch mean (partition-reduce via a ones
    matmul) are computed on every core; core 0's scalar is returned.

The Bacc lowering (`nc.compile()`) splits semaphore waits that exceed the
per-instruction ISA sync-slot limits; the explicit ldweights "absorber"
instructions and ACT/DVE-side copies keep hot-loop instructions within one
foreign wait each so that pass stays cheap.
"""

import numpy as np
from contextlib import ExitStack

B = 1024
D = 512
C = 100000
NCORES = 8
C_LOC = C // NCORES          # 12500
SCALE = 30.0
MARGIN = 0.5
SM = SCALE * MARGIN          # 15.0
CHUNK = 1536                 # classes per PSUM stats group (3 fp32 banks)

RSQRT_MAGIC = 0x5F3759DF


def _dve_rsqrt_scale(nc, pool, x, n, scale_out, bufs=12):
    """out = scale_out / sqrt(x), elementwise on a [128, n] fp32 SBUF tile.

    Quake initial guess + 2 Newton iterations on the vector engine (the
    scalar engine's Rsqrt is banned, and Sqrt lives in a different activation
    table set than Exp/Ln — switching sets costs ~2.7us each time).
    """
    import concourse.mybir as mybir

    f32 = mybir.dt.float32
    i32 = mybir.dt.int32
    y = pool.tile([128, n], f32, tag="rsq_y", bufs=bufs, name="rsq_y")
    t = pool.tile([128, n], f32, tag="rsq_t", bufs=bufs, name="rsq_t")
    xc = pool.tile([128, n], f32, tag="rsq_x", bufs=bufs, name="rsq_x")
    # clamp away from 0 (shard-padding rows have sum-of-squares 0; matches
    # the reference's max(norm, eps) in spirit)
    nc.vector.tensor_scalar(
        out=xc, in0=x, scalar1=1e-12, scalar2=None, op0=mybir.AluOpType.max)
    x = xc
    xi = x.bitcast(i32)
    yi = y.bitcast(i32)
    # y0 bits = MAGIC - (xi >> 1)  ==  (~(xi >> 1)) + (MAGIC + 1)
    nc.vector.tensor_scalar(
        out=yi, in0=xi, scalar1=1, scalar2=-1,
        op0=mybir.AluOpType.arith_shift_right, op1=mybir.AluOpType.bitwise_xor,
    )
    nc.vector.tensor_scalar(
        out=yi, in0=yi, scalar1=RSQRT_MAGIC + 1, scalar2=None,
        op0=mybir.AluOpType.add,
    )
    for it in range(2):
        nc.vector.tensor_tensor(out=t, in0=y, in1=y, op=mybir.AluOpType.mult)
        nc.vector.tensor_tensor(out=t, in0=t, in1=x, op=mybir.AluOpType.mult)
        # t = 1.5 - 0.5 * x * y^2
        nc.vector.tensor_scalar(
            out=t, in0=t, scalar1=-0.5, scalar2=1.5,
            op0=mybir.AluOpType.mult, op1=mybir.AluOpType.add,
        )
        if it == 1:
            nc.vector.tensor_scalar(
                out=t, in0=t, scalar1=float(scale_out), scalar2=None,
                op0=mybir.AluOpType.mult,
            )
        nc.vector.tensor_tensor(out=y, in0=y, in1=t, op=mybir.AluOpType.mult)
    return y


def build_nc(b=B, d=D, c_loc=C_LOC, chunk=CHUNK, n_cores=NCORES, debug=False):
    import concourse.bass as bass
    import concourse.tile as tile
    import concourse.mybir as mybir
    from concourse.masks import make_identity

    f32 = mybir.dt.float32
    bf16 = mybir.dt.bfloat16
    i32 = mybir.dt.int32
    Alu = mybir.AluOpType
    Act = mybir.ActivationFunctionType

    nb = b // 128            # batch tiles
    nd = d // 128            # contraction tiles
    c_pad = ((c_loc + 127) // 128) * 128
    groups = []
    c0 = 0
    while c0 < c_pad:
        gw = min(chunk, c_pad - c0)
        groups.append((c0, gw))
        c0 += gw
    ng = len(groups)

    if debug:
        nc = bass.Bass(target_bir_lowering=False, debug=True)
    else:
        from concourse import bacc
        nc = bacc.Bacc()

    # the weight shard arrives zero-padded to a multiple of 128 rows
    emb = nc.declare_dram_parameter("emb", [b, d], f32, isOutput=False)
    wsh = nc.declare_dram_parameter("wsh", [c_pad, d], f32, isOutput=False)
    lab = nc.declare_dram_parameter("lab", [b], i32, isOutput=False)
    out = nc.declare_dram_parameter("out", [1], f32, isOutput=True)

    with ExitStack() as ctx:
        tc = ctx.enter_context(tile.TileContext(nc))
        dram = ctx.enter_context(tc.tile_pool(name="dram", bufs=1, space="DRAM"))
        res = ctx.enter_context(tc.tile_pool(name="res", bufs=1))
        work = ctx.enter_context(tc.tile_pool(name="work", bufs=3))
        wtp = ctx.enter_context(tc.tile_pool(name="wtp", bufs=3))
        psum = ctx.enter_context(tc.tile_pool(name="psum", bufs=2, space="PSUM"))

        # collective bounce buffers (internal DRAM)
        st_in = dram.tile([128, 3 * nb], f32, tag="st_in", name="st_in")
        st_out = dram.tile([n_cores * 128, 3 * nb], f32, tag="st_out",
                           name="st_out", addr_space="Shared")

        ident = res.tile([128, 128], bf16, tag="ident", name="ident")
        make_identity(nc, ident)

        # ---------------- Phase A: embeddings ----------------
        e_all = res.tile([128, nb, d], f32, tag="e_all", name="e_all")
        nc.sync.dma_start(out=e_all, in_=emb.rearrange("(t p) d -> p t d", p=128))
        e_b16 = res.tile([128, nb, d], bf16, tag="e_b16", name="e_b16")
        nc.vector.tensor_copy(out=e_b16, in_=e_all)
        eT = res.tile([128, nd, b], bf16, tag="eT", name="eT")
        for t in range(nb):
            tp = psum.tile([128, d], bf16, tag="tp", name="tp", space="PSUM")
            # dummy weight-load absorbs the newest DVE tick (input data for
            # fresh slots, the recycled slot's last evacuation otherwise) on
            # the PE's clock, so the transposes below carry at most one
            # semaphore wait (the matmul ISA struct fits 1 wait + 1 update)
            if t < 2:
                nc.tensor.ldweights(e_b16[:, t, 0:128])
            else:
                nc.tensor.ldweights(eT[:, 0, (t - 2) * 128:(t - 1) * 128])
            for dd in range(nd):
                nc.tensor.transpose(
                    tp[:, dd * 128:(dd + 1) * 128],
                    e_b16[:, t, dd * 128:(dd + 1) * 128], ident)
            nc.vector.tensor_copy(
                out=eT[:, :, t * 128:(t + 1) * 128],
                in_=tp.rearrange("p (a q) -> p a q", a=nd))
        e_f = [e_all[:, t, :] for t in range(nb)]

        # ---------------- stats tiles ----------------
        nm_st = []
        nm2_st = []
        s_st = []
        for t in range(nb):
            nmt = res.tile([128, ng], f32, tag=f"nm{t}", name=f"nm{t}")
            nm2t = res.tile([128, ng], f32, tag=f"nm2_{t}", name=f"nm2_{t}")
            sst = res.tile([128, ng], f32, tag=f"ss{t}", name=f"ss{t}")
            nm_st.append(nmt)
            nm2_st.append(nm2t)
            s_st.append(sst)

        # ---------------- Phases B/C/D per group ----------------
        # tiny per-sub DVE probe reads absorb the load-DMA semaphore so the
        # normalize ops below only carry their (mandatory) same-engine wait
        probe_all = res.tile([128, 64], f32, tag="probe", name="probe_all")
        probe_idx = [0]
        SUB = 6  # w blocks per f32 staging tile (SBUF pressure)
        for g, (g0, gw) in enumerate(groups):
            nblk = gw // 128
            # B: load f32 sub-tiles, sum-of-squares (ACT Square+accum — the
            # junk elementwise output goes into the wtb slice that the
            # normalize overwrites later, so it adds no scratch and no WAW
            # semaphore), rsqrt (DVE), normalize+cast to bf16 (DVE).
            # Shard-padding rows are zero: squares 0, rsqrt clamped, w_n 0.
            wtb = work.tile([128, nblk, d], bf16, tag="wtb", bufs=3, name="wtb")
            for s0 in range(0, nblk, SUB):
                sblk = min(SUB, nblk - s0)
                wtg = work.tile([128, SUB, d], f32, tag="wtg", bufs=3, name="wtg")
                nc.sync.dma_start(
                    out=wtg[:, :sblk, :],
                    in_=wsh[g0 + s0 * 128:g0 + (s0 + sblk) * 128, :]
                    .rearrange("(k p) d -> p k d", p=128))
                pi = probe_idx[0]
                probe_idx[0] += 1
                assert pi < 64
                nc.vector.tensor_copy(out=probe_all[:, pi:pi + 1],
                                      in_=wtg[:, 0, 0:1])
                ssq = work.tile([128, SUB], f32, tag="ssq", bufs=4, name="ssq")
                for kk in range(sblk):
                    k = s0 + kk
                    nc.scalar.activation(
                        out=wtb[:, k, :], in_=wtg[:, kk, :], func=Act.Square,
                        accum_out=ssq[:, kk:kk + 1])
                # per-sub rsqrt: no group-wide barrier, each 6-block chain
                # flows into its transposes independently
                invs = _dve_rsqrt_scale(nc, work, ssq[:, :sblk], sblk, SCALE)
                for kk in range(sblk):
                    k = s0 + kk
                    nc.vector.tensor_scalar(
                        out=wtb[:, k, :], in0=wtg[:, kk, :],
                        scalar1=invs[:, kk:kk + 1], scalar2=None, op0=Alu.mult)

            # C: transpose each 128-class block on the tensor engine
            wT = wtp.tile([128, nd, gw], bf16, tag="wT", name="wT")
            for k in range(nblk):
                tp = psum.tile([128, d], bf16, tag="tp", name="tp", space="PSUM")
                # wait absorber: target the newest DVE product (see eT loop)
                if k < 2:
                    nc.tensor.ldweights(wtb[:, k, 0:128])
                else:
                    nc.tensor.ldweights(wT[:, 0, (k - 2) * 128:(k - 1) * 128])
                for dd in range(nd):
                    nc.tensor.transpose(
                        tp[:, dd * 128:(dd + 1) * 128],
                        wtb[:, k, dd * 128:(dd + 1) * 128], ident)
                nc.vector.tensor_copy(
                    out=wT[:, :, k * 128:(k + 1) * 128],
                    in_=tp.rearrange("p (a q) -> p a q", a=nd))

            # D: matmul + stats per batch tile
            nsub = (gw + 511) // 512
            for t in range(nb):
                pt = psum.tile([128, chunk], f32, tag="pt", name="pt",
                               space="PSUM")
                if t == 0:
                    # absorb the group's last evacuation tick
                    nc.tensor.ldweights(wT[:, 0, (nblk - 1) * 128:nblk * 128])
                for s in range(nsub):
                    n0 = s * 512
                    nw = min(512, gw - n0)
                    for dd in range(nd):
                        nc.tensor.matmul(
                            pt[:, n0:n0 + nw],
                            eT[:, dd, t * 128:(t + 1) * 128],
                            wT[:, dd, n0:n0 + nw],
                            start=(dd == 0), stop=(dd == nd - 1),
                        )
                nc.vector.tensor_reduce(
                    out=nm_st[t][:, g:g + 1], in_=pt[:, :gw],
                    axis=mybir.AxisListType.X, op=Alu.max, negate=True)
                # route the bias through an ACT-side copy so the Exp below
                # only has to semaphore-wait on the tensor engine (the ACT
                # ISA struct has a single usable wait slot); the copy also
                # makes ACT observe the DVE tick, absorbing the exp's WAR
                # on the reduce_max's PSUM read
                nc.scalar.copy(out=nm2_st[t][:, g:g + 1],
                               in_=nm_st[t][:, g:g + 1])
                nc.scalar.activation(
                    out=pt[:, :gw], in_=pt[:, :gw], func=Act.Exp,
                    bias=nm2_st[t][:, g:g + 1], scale=1.0,
                    accum_out=s_st[t][:, g:g + 1])

        # ---------------- Phase F: local combine + collectives ----------------
        m_all = res.tile([128, nb], f32, tag="m_all", name="m_all")
        mk_neg = res.tile([128, nb], f32, tag="mk_neg", name="mk_neg")
        s_all = res.tile([128, nb], f32, tag="s_all", name="s_all")
        for t in range(nb):
            # nm_st holds negated group maxes; local max m_k = -min(nm)
            nc.vector.tensor_reduce(
                out=mk_neg[:, t:t + 1], in_=nm_st[t],
                axis=mybir.AxisListType.X, op=Alu.min)
            nc.vector.tensor_reduce(
                out=m_all[:, t:t + 1], in_=nm_st[t],
                axis=mybir.AxisListType.X, op=Alu.min, negate=True)
            ex = work.tile([128, ng], f32, tag="ex", bufs=2, name="ex")
            # exp(m_g - m_k) = exp(-nm_g + mk_neg)
            nc.scalar.activation(
                out=ex, in_=nm_st[t], func=Act.Exp,
                bias=mk_neg[:, t:t + 1], scale=-1.0)
            nc.vector.tensor_tensor(out=ex, in0=ex, in1=s_st[t], op=Alu.mult)
            nc.vector.tensor_reduce(
                out=s_all[:, t:t + 1], in_=ex,
                axis=mybir.AxisListType.X, op=Alu.add)

        # ---------------- Phase E: label logits ----------------
        lab_s = res.tile([128, nb], i32, tag="lab_s", name="lab_s")
        nc.sync.dma_start(out=lab_s, in_=lab.rearrange("(t p) -> p t", p=128))
        labf = res.tile([128, nb], f32, tag="labf", name="labf")
        nc.vector.tensor_copy(out=labf, in_=lab_s)
        offs = res.tile([128, nb], i32, tag="offs", name="offs")
        nc.vector.tensor_scalar(
            out=offs, in0=lab_s, scalar1=0, scalar2=c_loc - 1,
            op0=Alu.max, op1=Alu.min)
        T_st = res.tile([128, nb], f32, tag="T_st", name="T_st")
        ssg = res.tile([128, nb], f32, tag="ssg", name="ssg")
        ldot = res.tile([128, nb], f32, tag="ldot", name="ldot")
        # one indirect DMA gathers all label rows (keeps SWDGE lanes fresh)
        wg_all = res.tile([128, nb, d], f32, tag="wg_all", name="wg_all")
        nc.gpsimd.indirect_dma_start(
            out=wg_all, out_offset=None, in_=wsh[:, :],
            in_offset=bass.IndirectOffsetOnAxis(ap=offs[:, :], axis=0),
        )
        lab_junk2 = res.tile([128, nb, d], bf16, tag="lab_junk2", name="lab_junk2")
        lab_junk = res.tile([128, nb, d], bf16, tag="lab_junk", name="lab_junk")
        for t in range(nb):
            nc.vector.tensor_tensor(
                out=lab_junk2[:, t, :], in0=wg_all[:, t, :], in1=e_f[t],
                op=Alu.mult)
            nc.vector.tensor_reduce(
                out=ldot[:, t:t + 1], in_=lab_junk2[:, t, :],
                axis=mybir.AxisListType.X, op=Alu.add)
        for t in range(nb):
            # elementwise output is junk (fresh bf16 slices, no hazards);
            # only the f32 accum matters
            nc.scalar.activation(
                out=lab_junk[:, t, :], in_=wg_all[:, t, :], func=Act.Square,
                accum_out=ssg[:, t:t + 1])
        linv = _dve_rsqrt_scale(nc, res, ssg, nb, SCALE)
        nc.vector.tensor_tensor(out=T_st, in0=ldot, in1=linv, op=Alu.mult)
        # mask out labels that don't belong to this core's shard
        msk = res.tile([128, nb], f32, tag="msk", name="msk")
        nc.vector.tensor_scalar(
            out=msk, in0=labf, scalar1=0.0, scalar2=None, op0=Alu.is_ge)
        nc.vector.tensor_tensor(out=T_st, in0=T_st, in1=msk, op=Alu.mult)
        nc.vector.tensor_scalar(
            out=msk, in0=labf, scalar1=float(c_loc - 1), scalar2=None,
            op0=Alu.is_le)
        nc.vector.tensor_tensor(out=T_st, in0=T_st, in1=msk, op=Alu.mult)


        # single AllGather of (m_k, s_k, t_k): output lands core-major on
        # the partition axis; the DMA back rearranges it so the cross-core
        # reduction becomes ordinary free-axis vector math
        stpack = res.tile([128, 3 * nb], f32, tag="stpack", name="stpack")
        nc.vector.tensor_copy(out=stpack[:, 0:nb], in_=m_all)
        nc.vector.tensor_copy(out=stpack[:, nb:2 * nb], in_=s_all)
        nc.vector.tensor_copy(out=stpack[:, 2 * nb:3 * nb], in_=T_st)
        nc.sync.dma_start(out=st_in[:, :], in_=stpack)
        nc.gpsimd.collective_compute(
            "AllGather", Alu.bypass,
            replica_groups=[list(range(n_cores))],
            ins=[st_in[:, :]], outs=[st_out[:, :]],
        )
        AG = res.tile([128, n_cores, 3 * nb], f32, tag="AG", name="AG")
        nc.sync.dma_start(
            out=AG, in_=st_out[:, :].rearrange("(k p) c -> p k c", p=128))
        AG2 = res.tile([128, n_cores, 3 * nb], f32, tag="AG2", name="AG2")
        nc.vector.tensor_copy(out=AG2, in_=AG)
        Mk = AG2[:, :, 0:nb]
        Sk = AG2[:, :, nb:2 * nb]
        Tk = AG2[:, :, 2 * nb:3 * nb]
        M2 = res.tile([128, nb], f32, tag="M2", name="M2")
        nc.vector.tensor_reduce(
            out=M2, in_=Mk.rearrange("p k c -> p c k"),
            axis=mybir.AxisListType.X, op=Alu.max)
        M2n = res.tile([128, nb], f32, tag="M2n", name="M2n")
        nc.vector.tensor_reduce(
            out=M2n, in_=Mk.rearrange("p k c -> p c k"),
            axis=mybir.AxisListType.X, op=Alu.max, negate=True)
        # s_k * exp(m_k - M), summed over cores
        dif = res.tile([128, n_cores, nb], f32, tag="dif", name="dif")
        for k in range(n_cores):
            nc.vector.tensor_tensor(
                out=dif[:, k, :], in0=Mk[:, k, :], in1=M2n, op=Alu.add)
        exd = res.tile([128, n_cores, nb], f32, tag="exd", name="exd")
        nc.scalar.activation(out=exd, in_=dif, func=Act.Exp)
        exd2 = res.tile([128, n_cores, nb], f32, tag="exd2", name="exd2")
        nc.vector.tensor_tensor(out=exd2, in0=exd, in1=Sk, op=Alu.mult)
        SGt = res.tile([128, nb], f32, tag="SGt", name="SGt")
        nc.vector.tensor_reduce(
            out=SGt, in_=exd2.rearrange("p k c -> p c k"),
            axis=mybir.AxisListType.X, op=Alu.add)
        TGt = res.tile([128, nb], f32, tag="TGt", name="TGt")
        nc.vector.tensor_reduce(
            out=TGt, in_=Tk.rearrange("p k c -> p c k"),
            axis=mybir.AxisListType.X, op=Alu.add)
        SG = SGt[:, :]
        TG = TGt[:, :]

        # loss_b = M + log(S - exp(T-M) + exp(T-M-SM)) - T + SM
        tmd = res.tile([128, nb], f32, tag="tmd", name="tmd")
        nc.vector.tensor_tensor(out=tmd, in0=TG, in1=M2, op=Alu.subtract)
        ea = res.tile([128, nb], f32, tag="ea", name="ea")
        nc.scalar.activation(out=ea, in_=tmd, func=Act.Exp)
        nsm = res.tile([128, 1], f32, tag="nsm", name="nsm")
        nc.vector.memset(nsm, -SM)
        eb = res.tile([128, nb], f32, tag="eb", name="eb")
        nc.scalar.activation(out=eb, in_=tmd, func=Act.Exp, bias=nsm[:, 0:1])
        ea2 = res.tile([128, nb], f32, tag="ea2", name="ea2")
        nc.vector.tensor_copy(out=ea2, in_=ea)
        eb2 = res.tile([128, nb], f32, tag="eb2", name="eb2")
        nc.vector.tensor_copy(out=eb2, in_=eb)
        S2 = res.tile([128, nb], f32, tag="S2", name="S2")
        nc.vector.tensor_tensor(out=S2, in0=SG, in1=ea2, op=Alu.subtract)
        nc.vector.tensor_tensor(out=S2, in0=S2, in1=eb2, op=Alu.add)
        lg = res.tile([128, nb], f32, tag="lg", name="lg")
        nc.scalar.activation(out=lg, in_=S2, func=Act.Ln)
        lg2 = res.tile([128, nb], f32, tag="lg2", name="lg2")
        nc.vector.tensor_copy(out=lg2, in_=lg)
        nc.vector.tensor_tensor(out=lg2, in0=lg2, in1=M2, op=Alu.add)
        nc.vector.tensor_tensor(out=lg2, in0=lg2, in1=TG, op=Alu.subtract)
        nc.vector.tensor_scalar(
            out=lg2, in0=lg2, scalar1=SM, scalar2=None, op0=Alu.add)

        rs = res.tile([128, 1], f32, tag="rs", name="rs")
        nc.vector.tensor_reduce(
            out=rs, in_=lg2, axis=mybir.AxisListType.X, op=Alu.add)
        # route rs through ACT so the final matmul's two deps (operand +
        # PSUM-slot release, both ACT) merge into a single semaphore wait;
        # the ones-vector is the pre-barrier const AP (no wait at all)
        rs2 = res.tile([128, 1], f32, tag="rs2", name="rs2")
        nc.scalar.copy(out=rs2, in_=rs)
        ones = nc.const_aps.tensor(1.0, (128, 1))
        fin = psum.tile([1, 1], f32, tag="pt", name="fin", space="PSUM")
        nc.tensor.matmul(fin, rs2, ones, start=True, stop=True)
        out_sb = res.tile([1, 1], f32, tag="out_sb", name="out_sb")
        nc.vector.tensor_scalar(
            out=out_sb, in0=fin[0:1, 0:1], scalar1=1.0 / b, scalar2=None,
            op0=Alu.mult)
        # SWDGE store on a fresh lane: carries only the DVE data wait
        nc.gpsimd.dma_start(out=out[0:1], in_=out_sb[0:1, 0])

    if not debug:
        # bacc lowering: splits over-capacity semaphore waits into
        # event-semaphore instructions (hardware sync-slot limits)
        nc.compile()
    return nc


def kernel(embeddings, labels, weight):
    import concourse.bass_utils as bass_utils

    emb = np.ascontiguousarray(np.asarray(embeddings, dtype=np.float32))
    labv = np.asarray(labels).astype(np.int64)
    w = np.asarray(weight, dtype=np.float32)

    nc = build_nc()
    c_pad = ((C_LOC + 127) // 128) * 128
    in_maps = []
    for k in range(NCORES):
        wpad = np.zeros((c_pad, D), dtype=np.float32)
        wpad[:C_LOC] = w[k * C_LOC:(k + 1) * C_LOC]
        in_maps.append({
            "emb": emb,
            "wsh": wpad,
            "lab": (labv - k * C_LOC).astype(np.int32),
        })
    res = bass_utils.run_bass_kernel_spmd(nc, in_maps, core_ids=list(range(NCORES)))
    return np.float32(np.asarray(res.results[0]["out"]).ravel()[0])


# revision 55
# speedup vs baseline: 1.0387x; 1.0117x over previous
"""ArcFace loss kernel for 8 TRN2 NeuronCores.

Reference computation (see problem statement):
    w_n   = weight / max(||weight_row||, 1e-12)            # [C, D]
    cos   = emb @ w_n.T                                    # [B, C]
    logit = SCALE * cos;  logit[b, lab[b]] -= SCALE*MARGIN
    loss  = mean_b( logsumexp(logit[b]) - logit[b, lab[b]] )

Sharding: classes (C=100000) split over 8 cores (12500 each), embeddings +
labels replicated.  Per core, fused in one NEFF:
  - weight shard is streamed in, row-normalized (sum-of-squares on the scalar
    engine via Square+accum, rsqrt via quake-trick + Newton on the vector
    engine, SCALE folded in), cast to bf16 on gpsimd, transposed 128x128 at a
    time on the tensor engine to produce w_n^T tiles for the matmul,
  - logits are accumulated in PSUM groups of [128b x 1536c] (bf16 matmul,
    fp32 accumulate); per-group row-max on the vector engine (negated, to
    serve as the exp bias), exp + row-sum in one scalar-engine op reading
    PSUM in place,
  - label logits come from an indirect-DMA row gather of the weight shard,
  - cross-core softmax reduction is one small AllGather of per-core
    (row-max, row-sumexp, label-logit) stats; every core redundantly
    combines them with free-axis vector math,
  - final per-row loss and the batch mean (partition-reduce via a ones
    matmul) are computed on every core; core 0's scalar is returned.

The Bacc lowering (`nc.compile()`) splits semaphore waits that exceed the
per-instruction ISA sync-slot limits; the explicit ldweights "absorber"
instructions and ACT/DVE-side copies keep hot-loop instructions within one
foreign wait each so that pass stays cheap.
"""

import numpy as np
from contextlib import ExitStack

B = 1024
D = 512
C = 100000
NCORES = 8
C_LOC = C // NCORES          # 12500
SCALE = 30.0
MARGIN = 0.5
SM = SCALE * MARGIN          # 15.0
CHUNK = 1536                 # classes per PSUM stats group (3 fp32 banks)

RSQRT_MAGIC = 0x5F3759DF


def _dve_rsqrt_scale(nc, pool, x, n, scale_out, bufs=12):
    """out = scale_out / sqrt(x), elementwise on a [128, n] fp32 SBUF tile.

    Quake initial guess + 2 Newton iterations on the vector engine (the
    scalar engine's Rsqrt is banned, and Sqrt lives in a different activation
    table set than Exp/Ln — switching sets costs ~2.7us each time).
    """
    import concourse.mybir as mybir

    f32 = mybir.dt.float32
    i32 = mybir.dt.int32
    y = pool.tile([128, n], f32, tag="rsq_y", bufs=bufs, name="rsq_y")
    t = pool.tile([128, n], f32, tag="rsq_t", bufs=bufs, name="rsq_t")
    xc = pool.tile([128, n], f32, tag="rsq_x", bufs=bufs, name="rsq_x")
    # clamp away from 0 (shard-padding rows have sum-of-squares 0; matches
    # the reference's max(norm, eps) in spirit)
    nc.vector.tensor_scalar(
        out=xc, in0=x, scalar1=1e-12, scalar2=None, op0=mybir.AluOpType.max)
    x = xc
    xi = x.bitcast(i32)
    yi = y.bitcast(i32)
    # y0 bits = MAGIC - (xi >> 1)  ==  (~(xi >> 1)) + (MAGIC + 1)
    nc.vector.tensor_scalar(
        out=yi, in0=xi, scalar1=1, scalar2=-1,
        op0=mybir.AluOpType.arith_shift_right, op1=mybir.AluOpType.bitwise_xor,
    )
    nc.vector.tensor_scalar(
        out=yi, in0=yi, scalar1=RSQRT_MAGIC + 1, scalar2=None,
        op0=mybir.AluOpType.add,
    )
    for it in range(2):
        nc.vector.tensor_tensor(out=t, in0=y, in1=y, op=mybir.AluOpType.mult)
        nc.vector.tensor_tensor(out=t, in0=t, in1=x, op=mybir.AluOpType.mult)
        # t = 1.5 - 0.5 * x * y^2
        nc.vector.tensor_scalar(
            out=t, in0=t, scalar1=-0.5, scalar2=1.5,
            op0=mybir.AluOpType.mult, op1=mybir.AluOpType.add,
        )
        if it == 1:
            nc.vector.tensor_scalar(
                out=t, in0=t, scalar1=float(scale_out), scalar2=None,
                op0=mybir.AluOpType.mult,
            )
        nc.vector.tensor_tensor(out=y, in0=y, in1=t, op=mybir.AluOpType.mult)
    return y


def build_nc(b=B, d=D, c_loc=C_LOC, chunk=CHUNK, n_cores=NCORES, debug=False):
    import concourse.bass as bass
    import concourse.tile as tile
    import concourse.mybir as mybir
    from concourse.masks import make_identity

    f32 = mybir.dt.float32
    bf16 = mybir.dt.bfloat16
    i32 = mybir.dt.int32
    Alu = mybir.AluOpType
    Act = mybir.ActivationFunctionType

    nb = b // 128            # batch tiles
    nd = d // 128            # contraction tiles
    c_pad = ((c_loc + 127) // 128) * 128
    groups = []
    c0 = 0
    while c0 < c_pad:
        gw = min(chunk, c_pad - c0)
        groups.append((c0, gw))
        c0 += gw
    ng = len(groups)

    if debug:
        nc = bass.Bass(target_bir_lowering=False, debug=True)
    else:
        from concourse import bacc
        nc = bacc.Bacc()

    # the weight shard arrives zero-padded to a multiple of 128 rows
    emb = nc.declare_dram_parameter("emb", [b, d], f32, isOutput=False)
    wsh = nc.declare_dram_parameter("wsh", [c_pad, d], f32, isOutput=False)
    lab = nc.declare_dram_parameter("lab", [b], i32, isOutput=False)
    out = nc.declare_dram_parameter("out", [1], f32, isOutput=True)

    with ExitStack() as ctx:
        tc = ctx.enter_context(tile.TileContext(nc))
        dram = ctx.enter_context(tc.tile_pool(name="dram", bufs=1, space="DRAM"))
        res = ctx.enter_context(tc.tile_pool(name="res", bufs=1))
        work = ctx.enter_context(tc.tile_pool(name="work", bufs=3))
        wtp = ctx.enter_context(tc.tile_pool(name="wtp", bufs=3))
        psum = ctx.enter_context(tc.tile_pool(name="psum", bufs=2, space="PSUM"))

        # collective bounce buffers (internal DRAM)
        st_in = dram.tile([128, 3 * nb], f32, tag="st_in", name="st_in")
        st_out = dram.tile([n_cores * 128, 3 * nb], f32, tag="st_out",
                           name="st_out", addr_space="Shared")

        ident = res.tile([128, 128], bf16, tag="ident", name="ident")
        make_identity(nc, ident)

        # ---------------- Phase A: embeddings ----------------
        e_all = res.tile([128, nb, d], f32, tag="e_all", name="e_all")
        nc.sync.dma_start(out=e_all, in_=emb.rearrange("(t p) d -> p t d", p=128))
        e_b16 = res.tile([128, nb, d], bf16, tag="e_b16", name="e_b16")
        nc.vector.tensor_copy(out=e_b16, in_=e_all)
        eT = res.tile([128, nd, b], bf16, tag="eT", name="eT")
        for t in range(nb):
            tp = psum.tile([128, d], bf16, tag="tp", name="tp", space="PSUM")
            # dummy weight-load absorbs the newest DVE tick (input data for
            # fresh slots, the recycled slot's last evacuation otherwise) on
            # the PE's clock, so the transposes below carry at most one
            # semaphore wait (the matmul ISA struct fits 1 wait + 1 update)
            if t < 2:
                nc.tensor.ldweights(e_b16[:, t, 0:128])
            else:
                nc.tensor.ldweights(eT[:, 0, (t - 2) * 128:(t - 1) * 128])
            for dd in range(nd):
                nc.tensor.transpose(
                    tp[:, dd * 128:(dd + 1) * 128],
                    e_b16[:, t, dd * 128:(dd + 1) * 128], ident)
            nc.vector.tensor_copy(
                out=eT[:, :, t * 128:(t + 1) * 128],
                in_=tp.rearrange("p (a q) -> p a q", a=nd))
        e_f = [e_all[:, t, :] for t in range(nb)]

        # ---------------- stats tiles ----------------
        nm_st = []
        nm2_st = []
        s_st = []
        for t in range(nb):
            nmt = res.tile([128, ng], f32, tag=f"nm{t}", name=f"nm{t}")
            nm2t = res.tile([128, ng], f32, tag=f"nm2_{t}", name=f"nm2_{t}")
            sst = res.tile([128, ng], f32, tag=f"ss{t}", name=f"ss{t}")
            nm_st.append(nmt)
            nm2_st.append(nm2t)
            s_st.append(sst)

        # ---------------- Phases B/C/D per group ----------------
        # tiny per-sub DVE probe reads absorb the load-DMA semaphore so the
        # normalize ops below only carry their (mandatory) same-engine wait
        probe_all = res.tile([128, 64], f32, tag="probe", name="probe_all")
        probe_idx = [0]
        SUB = 6  # w blocks per f32 staging tile (SBUF pressure)
        for g, (g0, gw) in enumerate(groups):
            nblk = gw // 128
            # B: load f32 sub-tiles, sum-of-squares (ACT Square+accum — the
            # junk elementwise output goes into the wtb slice that the
            # normalize overwrites later, so it adds no scratch and no WAW
            # semaphore), rsqrt (DVE), normalize+cast to bf16 (DVE).
            # Shard-padding rows are zero: squares 0, rsqrt clamped, w_n 0.
            wtb = work.tile([128, nblk, d], bf16, tag="wtb", bufs=4, name="wtb")
            for s0 in range(0, nblk, SUB):
                sblk = min(SUB, nblk - s0)
                wtg = work.tile([128, SUB, d], f32, tag="wtg", bufs=3, name="wtg")
                nc.sync.dma_start(
                    out=wtg[:, :sblk, :],
                    in_=wsh[g0 + s0 * 128:g0 + (s0 + sblk) * 128, :]
                    .rearrange("(k p) d -> p k d", p=128))
                pi = probe_idx[0]
                probe_idx[0] += 1
                assert pi < 64
                nc.vector.tensor_copy(out=probe_all[:, pi:pi + 1],
                                      in_=wtg[:, 0, 0:1])
                ssq = work.tile([128, SUB], f32, tag="ssq", bufs=4, name="ssq")
                for kk in range(sblk):
                    k = s0 + kk
                    nc.scalar.activation(
                        out=wtb[:, k, :], in_=wtg[:, kk, :], func=Act.Square,
                        accum_out=ssq[:, kk:kk + 1])
                # per-sub rsqrt: no group-wide barrier, each 6-block chain
                # flows into its transposes independently
                invs = _dve_rsqrt_scale(nc, work, ssq[:, :sblk], sblk, SCALE)
                for kk in range(sblk):
                    k = s0 + kk
                    nc.vector.tensor_scalar(
                        out=wtb[:, k, :], in0=wtg[:, kk, :],
                        scalar1=invs[:, kk:kk + 1], scalar2=None, op0=Alu.mult)

            # C: transpose each 128-class block on the tensor engine
            wT = wtp.tile([128, nd, gw], bf16, tag="wT", name="wT")
            for k in range(nblk):
                tp = psum.tile([128, d], bf16, tag="tp", name="tp", space="PSUM")
                # wait absorber: target the newest DVE product (see eT loop)
                if k < 2:
                    nc.tensor.ldweights(wtb[:, k, 0:128])
                else:
                    nc.tensor.ldweights(wT[:, 0, (k - 2) * 128:(k - 1) * 128])
                for dd in range(nd):
                    nc.tensor.transpose(
                        tp[:, dd * 128:(dd + 1) * 128],
                        wtb[:, k, dd * 128:(dd + 1) * 128], ident)
                nc.vector.tensor_copy(
                    out=wT[:, :, k * 128:(k + 1) * 128],
                    in_=tp.rearrange("p (a q) -> p a q", a=nd))

            # D: matmul + stats per batch tile
            nsub = (gw + 511) // 512
            for t in range(nb):
                pt = psum.tile([128, chunk], f32, tag="pt", name="pt",
                               space="PSUM")
                if t == 0:
                    # absorb the group's last evacuation tick
                    nc.tensor.ldweights(wT[:, 0, (nblk - 1) * 128:nblk * 128])
                for s in range(nsub):
                    n0 = s * 512
                    nw = min(512, gw - n0)
                    for dd in range(nd):
                        nc.tensor.matmul(
                            pt[:, n0:n0 + nw],
                            eT[:, dd, t * 128:(t + 1) * 128],
                            wT[:, dd, n0:n0 + nw],
                            start=(dd == 0), stop=(dd == nd - 1),
                        )
                nc.vector.tensor_reduce(
                    out=nm_st[t][:, g:g + 1], in_=pt[:, :gw],
                    axis=mybir.AxisListType.X, op=Alu.max, negate=True)
                # route the bias through an ACT-side copy so the Exp below
                # only has to semaphore-wait on the tensor engine (the ACT
                # ISA struct has a single usable wait slot); the copy also
                # makes ACT observe the DVE tick, absorbing the exp's WAR
                # on the reduce_max's PSUM read
                nc.scalar.copy(out=nm2_st[t][:, g:g + 1],
                               in_=nm_st[t][:, g:g + 1])
                nc.scalar.activation(
                    out=pt[:, :gw], in_=pt[:, :gw], func=Act.Exp,
                    bias=nm2_st[t][:, g:g + 1], scale=1.0,
                    accum_out=s_st[t][:, g:g + 1])

        # ---------------- Phase F: local combine + collectives ----------------
        m_all = res.tile([128, nb], f32, tag="m_all", name="m_all")
        mk_neg = res.tile([128, nb], f32, tag="mk_neg", name="mk_neg")
        s_all = res.tile([128, nb], f32, tag="s_all", name="s_all")
        for t in range(nb):
            # nm_st holds negated group maxes; local max m_k = -min(nm)
            nc.vector.tensor_reduce(
                out=mk_neg[:, t:t + 1], in_=nm_st[t],
                axis=mybir.AxisListType.X, op=Alu.min)
            nc.vector.tensor_reduce(
                out=m_all[:, t:t + 1], in_=nm_st[t],
                axis=mybir.AxisListType.X, op=Alu.min, negate=True)
            ex = work.tile([128, ng], f32, tag="ex", bufs=2, name="ex")
            # exp(m_g - m_k) = exp(-nm_g + mk_neg)
            nc.scalar.activation(
                out=ex, in_=nm_st[t], func=Act.Exp,
                bias=mk_neg[:, t:t + 1], scale=-1.0)
            nc.vector.tensor_tensor(out=ex, in0=ex, in1=s_st[t], op=Alu.mult)
            nc.vector.tensor_reduce(
                out=s_all[:, t:t + 1], in_=ex,
                axis=mybir.AxisListType.X, op=Alu.add)

        # ---------------- Phase E: label logits ----------------
        lab_s = res.tile([128, nb], i32, tag="lab_s", name="lab_s")
        nc.sync.dma_start(out=lab_s, in_=lab.rearrange("(t p) -> p t", p=128))
        labf = res.tile([128, nb], f32, tag="labf", name="labf")
        nc.vector.tensor_copy(out=labf, in_=lab_s)
        offs = res.tile([128, nb], i32, tag="offs", name="offs")
        nc.vector.tensor_scalar(
            out=offs, in0=lab_s, scalar1=0, scalar2=c_loc - 1,
            op0=Alu.max, op1=Alu.min)
        T_st = res.tile([128, nb], f32, tag="T_st", name="T_st")
        ssg = res.tile([128, nb], f32, tag="ssg", name="ssg")
        ldot = res.tile([128, nb], f32, tag="ldot", name="ldot")
        # one indirect DMA gathers all label rows (keeps SWDGE lanes fresh)
        wg_all = res.tile([128, nb, d], f32, tag="wg_all", name="wg_all")
        nc.gpsimd.indirect_dma_start(
            out=wg_all, out_offset=None, in_=wsh[:, :],
            in_offset=bass.IndirectOffsetOnAxis(ap=offs[:, :], axis=0),
        )
        lab_junk2 = res.tile([128, nb, d], bf16, tag="lab_junk2", name="lab_junk2")
        lab_junk = res.tile([128, nb, d], bf16, tag="lab_junk", name="lab_junk")
        for t in range(nb):
            nc.vector.tensor_tensor(
                out=lab_junk2[:, t, :], in0=wg_all[:, t, :], in1=e_f[t],
                op=Alu.mult)
            nc.vector.tensor_reduce(
                out=ldot[:, t:t + 1], in_=lab_junk2[:, t, :],
                axis=mybir.AxisListType.X, op=Alu.add)
        for t in range(nb):
            # elementwise output is junk (fresh bf16 slices, no hazards);
            # only the f32 accum matters
            nc.scalar.activation(
                out=lab_junk[:, t, :], in_=wg_all[:, t, :], func=Act.Square,
                accum_out=ssg[:, t:t + 1])
        linv = _dve_rsqrt_scale(nc, res, ssg, nb, SCALE)
        nc.vector.tensor_tensor(out=T_st, in0=ldot, in1=linv, op=Alu.mult)
        # mask out labels that don't belong to this core's shard
        msk = res.tile([128, nb], f32, tag="msk", name="msk")
        nc.vector.tensor_scalar(
            out=msk, in0=labf, scalar1=0.0, scalar2=None, op0=Alu.is_ge)
        nc.vector.tensor_tensor(out=T_st, in0=T_st, in1=msk, op=Alu.mult)
        nc.vector.tensor_scalar(
            out=msk, in0=labf, scalar1=float(c_loc - 1), scalar2=None,
            op0=Alu.is_le)
        nc.vector.tensor_tensor(out=T_st, in0=T_st, in1=msk, op=Alu.mult)


        # single AllGather of (m_k, s_k, t_k): output lands core-major on
        # the partition axis; the DMA back rearranges it so the cross-core
        # reduction becomes ordinary free-axis vector math
        stpack = res.tile([128, 3 * nb], f32, tag="stpack", name="stpack")
        nc.vector.tensor_copy(out=stpack[:, 0:nb], in_=m_all)
        nc.vector.tensor_copy(out=stpack[:, nb:2 * nb], in_=s_all)
        nc.vector.tensor_copy(out=stpack[:, 2 * nb:3 * nb], in_=T_st)
        nc.sync.dma_start(out=st_in[:, :], in_=stpack)
        nc.gpsimd.collective_compute(
            "AllGather", Alu.bypass,
            replica_groups=[list(range(n_cores))],
            ins=[st_in[:, :]], outs=[st_out[:, :]],
        )
        AG = res.tile([128, n_cores, 3 * nb], f32, tag="AG", name="AG")
        nc.sync.dma_start(
            out=AG, in_=st_out[:, :].rearrange("(k p) c -> p k c", p=128))
        AG2 = res.tile([128, n_cores, 3 * nb], f32, tag="AG2", name="AG2")
        nc.vector.tensor_copy(out=AG2, in_=AG)
        Mk = AG2[:, :, 0:nb]
        Sk = AG2[:, :, nb:2 * nb]
        Tk = AG2[:, :, 2 * nb:3 * nb]
        M2 = res.tile([128, nb], f32, tag="M2", name="M2")
        nc.vector.tensor_reduce(
            out=M2, in_=Mk.rearrange("p k c -> p c k"),
            axis=mybir.AxisListType.X, op=Alu.max)
        M2n = res.tile([128, nb], f32, tag="M2n", name="M2n")
        nc.vector.tensor_reduce(
            out=M2n, in_=Mk.rearrange("p k c -> p c k"),
            axis=mybir.AxisListType.X, op=Alu.max, negate=True)
        # s_k * exp(m_k - M), summed over cores
        dif = res.tile([128, n_cores, nb], f32, tag="dif", name="dif")
        for k in range(n_cores):
            nc.vector.tensor_tensor(
                out=dif[:, k, :], in0=Mk[:, k, :], in1=M2n, op=Alu.add)
        exd = res.tile([128, n_cores, nb], f32, tag="exd", name="exd")
        nc.scalar.activation(out=exd, in_=dif, func=Act.Exp)
        exd2 = res.tile([128, n_cores, nb], f32, tag="exd2", name="exd2")
        nc.vector.tensor_tensor(out=exd2, in0=exd, in1=Sk, op=Alu.mult)
        SGt = res.tile([128, nb], f32, tag="SGt", name="SGt")
        nc.vector.tensor_reduce(
            out=SGt, in_=exd2.rearrange("p k c -> p c k"),
            axis=mybir.AxisListType.X, op=Alu.add)
        TGt = res.tile([128, nb], f32, tag="TGt", name="TGt")
        nc.vector.tensor_reduce(
            out=TGt, in_=Tk.rearrange("p k c -> p c k"),
            axis=mybir.AxisListType.X, op=Alu.add)
        SG = SGt[:, :]
        TG = TGt[:, :]

        # loss_b = M + log(S - exp(T-M) + exp(T-M-SM)) - T + SM
        tmd = res.tile([128, nb], f32, tag="tmd", name="tmd")
        nc.vector.tensor_tensor(out=tmd, in0=TG, in1=M2, op=Alu.subtract)
        ea = res.tile([128, nb], f32, tag="ea", name="ea")
        nc.scalar.activation(out=ea, in_=tmd, func=Act.Exp)
        nsm = res.tile([128, 1], f32, tag="nsm", name="nsm")
        nc.vector.memset(nsm, -SM)
        eb = res.tile([128, nb], f32, tag="eb", name="eb")
        nc.scalar.activation(out=eb, in_=tmd, func=Act.Exp, bias=nsm[:, 0:1])
        ea2 = res.tile([128, nb], f32, tag="ea2", name="ea2")
        nc.vector.tensor_copy(out=ea2, in_=ea)
        eb2 = res.tile([128, nb], f32, tag="eb2", name="eb2")
        nc.vector.tensor_copy(out=eb2, in_=eb)
        S2 = res.tile([128, nb], f32, tag="S2", name="S2")
        nc.vector.tensor_tensor(out=S2, in0=SG, in1=ea2, op=Alu.subtract)
        nc.vector.tensor_tensor(out=S2, in0=S2, in1=eb2, op=Alu.add)
        lg = res.tile([128, nb], f32, tag="lg", name="lg")
        nc.scalar.activation(out=lg, in_=S2, func=Act.Ln)
        lg2 = res.tile([128, nb], f32, tag="lg2", name="lg2")
        nc.vector.tensor_copy(out=lg2, in_=lg)
        nc.vector.tensor_tensor(out=lg2, in0=lg2, in1=M2, op=Alu.add)
        nc.vector.tensor_tensor(out=lg2, in0=lg2, in1=TG, op=Alu.subtract)
        nc.vector.tensor_scalar(
            out=lg2, in0=lg2, scalar1=SM, scalar2=None, op0=Alu.add)

        rs = res.tile([128, 1], f32, tag="rs", name="rs")
        nc.vector.tensor_reduce(
            out=rs, in_=lg2, axis=mybir.AxisListType.X, op=Alu.add)
        # route rs through ACT so the final matmul's two deps (operand +
        # PSUM-slot release, both ACT) merge into a single semaphore wait;
        # the ones-vector is the pre-barrier const AP (no wait at all)
        rs2 = res.tile([128, 1], f32, tag="rs2", name="rs2")
        nc.scalar.copy(out=rs2, in_=rs)
        ones = nc.const_aps.tensor(1.0, (128, 1))
        fin = psum.tile([1, 1], f32, tag="pt", name="fin", space="PSUM")
        nc.tensor.matmul(fin, rs2, ones, start=True, stop=True)
        out_sb = res.tile([1, 1], f32, tag="out_sb", name="out_sb")
        nc.vector.tensor_scalar(
            out=out_sb, in0=fin[0:1, 0:1], scalar1=1.0 / b, scalar2=None,
            op0=Alu.mult)
        # SWDGE store on a fresh lane: carries only the DVE data wait
        nc.gpsimd.dma_start(out=out[0:1], in_=out_sb[0:1, 0])

    if not debug:
        # bacc lowering: splits over-capacity semaphore waits into
        # event-semaphore instructions (hardware sync-slot limits)
        nc.compile()
    return nc


def kernel(embeddings, labels, weight):
    import concourse.bass_utils as bass_utils

    emb = np.ascontiguousarray(np.asarray(embeddings, dtype=np.float32))
    labv = np.asarray(labels).astype(np.int64)
    w = np.asarray(weight, dtype=np.float32)

    nc = build_nc()
    c_pad = ((C_LOC + 127) // 128) * 128
    in_maps = []
    for k in range(NCORES):
        wpad = np.zeros((c_pad, D), dtype=np.float32)
        wpad[:C_LOC] = w[k * C_LOC:(k + 1) * C_LOC]
        in_maps.append({
            "emb": emb,
            "wsh": wpad,
            "lab": (labv - k * C_LOC).astype(np.int32),
        })
    res = bass_utils.run_bass_kernel_spmd(nc, in_maps, core_ids=list(range(NCORES)))
    return np.float32(np.asarray(res.results[0]["out"]).ravel()[0])
